# revision 10
# baseline (speedup 1.0000x reference)
"""DBN-Sigma whitening (group-wise decorrelated batch norm), fused
single-pass kernel on 8 trn2 cores.

Strategy (data-parallel over batch N, all-reduce of per-group stats):
  Each core takes 8 of 64 images.  X is host-cast to bf16 in
  channel-major layout ([256, 8*3136] per core, 12.5KB-contiguous DMA
  rows) and streamed into SBUF once, staying RESIDENT (100KB/partition)
  for the whole kernel.  Per 128-channel half: m-chunks are transposed
  ([c,m]->[m,c]) on the PE (bf16 transpose -> PSUM -> V/S copy) and the
  raw second moment S2 = sum_m x x^T accumulates in PSUM via bf16
  matmuls.  S2 of each half is AllReduce'd across the 8 cores (64KB,
  DRAM bounce) as soon as that half's accumulation ends, overlapping
  with the other half's streaming/compute; a tiny warmup AllReduce at
  kernel start absorbs the collective channel's setup latency and
  aligns the cores while the input is still streaming.  sigma is formed
  on device from host-supplied exact-mean correction constants
  (sigma = S2*mask/M - mu mu^T*mask + eps I) and sigma^{-1/2} comes
  from 2 coupled Newton-Schulz iterations in f32 on the PE (sigma's
  eigenvalues are within 2% of 1, so NS matches eigh to ~2e-7 —
  this removes the host eigh of the 2-launch version and with it the
  second read of X).  weight is folded into the whitening matrix on
  device (wm @ diag(w)); whitening runs from resident SBUF data (bf16
  matmuls, bf16 PSUM), the shift (bias - w*wm@mu) is added during the
  PSUM->SBUF move on alternating V/S engines, and the output streams
  out in bf16 channel-major (host upcasts/reorders).

HBM traffic/core: 12.85MB in + 12.85MB out (vs 64MB for the 2-launch
f32 version).  Measured numerics: rel err ~7e-3 (tolerance 2e-2).
"""

import numpy as np
import ml_dtypes
import concourse.bass as bass
import concourse.bacc as bacc
import concourse.mybir as mybir
import concourse.tile as tile
from concourse.bass_utils import run_bass_kernel_spmd

N_CORES = 8
N, C, H, W = 64, 256, 56, 56
HW = H * W                     # 3136
NL = N // N_CORES              # 8 images per core
G, CG = 16, 16
EPS = 1e-3
M_TOT = N * HW
MC = NL * HW                   # 25088 pixels per core
FP = mybir.dt.float32
BF = mybir.dt.bfloat16

NP_ = NL // 2                  # 4 image pairs per core
FPAIR = 2 * HW                 # 6272 free elems per (pair, half) unit
NCH = FPAIR // 128             # 49 m-chunks per unit
NQG = 7                        # transpose chunks per PSUM group (49 = 7*7)
KT = 448                       # whiten matmul free-dim tile (14 * 448 = 6272)
NK = FPAIR // KT
NS_K = 2                       # Newton-Schulz iterations


def _build():
    nc = bacc.Bacc("TRN2", target_bir_lowering=False, debug=False,
                   num_devices=N_CORES)
    X_d = nc.dram_tensor("X", [C, MC], BF, kind="ExternalInput")
    eye_d = nc.dram_tensor("eye", [128, 128], BF, kind="ExternalInput")
    maskM_d = nc.dram_tensor("maskM", [128, 128], FP, kind="ExternalInput")
    corr_d = nc.dram_tensor("corr", [2, 128, 128], FP, kind="ExternalInput")
    i15_d = nc.dram_tensor("i15", [128, 128], FP, kind="ExternalInput")
    diagW_d = nc.dram_tensor("diagW", [2, 128, 128], FP, kind="ExternalInput")
    mub_d = nc.dram_tensor("mub", [128, 2], BF, kind="ExternalInput")
    bv_d = nc.dram_tensor("bv", [128, 2], FP, kind="ExternalInput")
    Xn_d = nc.dram_tensor("Xn", [C, MC], BF, kind="ExternalOutput")
    X = X_d.ap()
    Xn = Xn_d.ap()

    with tile.TileContext(nc) as tc:
        with (
            tc.tile_pool(name="const", bufs=1) as constp,
            tc.tile_pool(name="xres", bufs=1) as xresp,
            tc.tile_pool(name="xtq", bufs=6) as xtqp,
            tc.tile_pool(name="ns", bufs=1) as nsp,
            tc.tile_pool(name="obuf", bufs=3) as obufp,
            tc.tile_pool(name="ptp", bufs=3, space="PSUM") as ptp,
            tc.tile_pool(name="cov", bufs=1, space="PSUM") as covp,
            tc.tile_pool(name="nsps", bufs=1, space="PSUM") as nspsp,
            tc.tile_pool(name="wps", bufs=2, space="PSUM") as wpsp,
            tc.tile_pool(name="dram", bufs=1, space="DRAM") as dramp,
        ):
            # warmup AllReduce: pays the collective-channel setup cost and
            # aligns the 8 cores while the input DMA is still streaming
            warm = nsp.tile([1, 16], FP, name="warm")
            nc.vector.memset(warm[:], 1.0)
            winb = dramp.tile([1, 16], FP, name="winb")
            woutb = dramp.tile([1, 16], FP, name="woutb")
            nc.gpsimd.dma_start(winb[:], warm[:])
            nc.gpsimd.collective_compute(
                "AllReduce", mybir.AluOpType.add,
                replica_groups=[list(range(N_CORES))],
                ins=[winb[:].opt()], outs=[woutb[:].opt()],
            )

            eye = constp.tile([128, 128], BF)
            nc.sync.dma_start(eye[:], eye_d.ap())
            maskM = constp.tile([128, 128], FP)
            nc.sync.dma_start(maskM[:], maskM_d.ap())
            corr = [constp.tile([128, 128], FP, name=f"corr{h}") for h in (0, 1)]
            for h in (0, 1):
                nc.sync.dma_start(corr[h][:], corr_d.ap()[h])
            i15 = constp.tile([128, 128], FP)
            nc.sync.dma_start(i15[:], i15_d.ap())
            diagW = [constp.tile([128, 128], FP, name=f"diagW{h}")
                     for h in (0, 1)]
            for h in (0, 1):
                nc.sync.dma_start(diagW[h][:], diagW_d.ap()[h])
            mub = constp.tile([128, 2], BF)
            nc.sync.dma_start(mub[:], mub_d.ap())
            bv = constp.tile([128, 2], FP)
            nc.sync.dma_start(bv[:], bv_d.ap())

            xres = [xresp.tile([128, NP_ * FPAIR], BF, name=f"xres{h}")
                    for h in (0, 1)]
            cov = [covp.tile([128, 128], FP, name=f"cov{h}") for h in (0, 1)]
            covsb = [nsp.tile([128, 128], FP, name=f"covsb{h}") for h in (0, 1)]
            s2r = [nsp.tile([128, 128], FP, name=f"s2r{h}") for h in (0, 1)]
            inb = [dramp.tile([128, 128], FP, name=f"inb{h}") for h in (0, 1)]
            outb = [dramp.tile([128, 128], FP, name=f"outb{h}") for h in (0, 1)]

            # ---------------- Phase 1: stream in, accumulate S2 ----------
            # PSUM->SBUF copies alternate V/S per chunk-group so both
            # engines drain the PE's transposes concurrently.
            qeng = 0
            for h in (0, 1):
                started = False
                for p in range(NP_):
                    xs = xres[h][:, FPAIR * p:FPAIR * (p + 1)]
                    nc.sync.dma_start(
                        xs, X[128 * h:128 * (h + 1),
                              FPAIR * p:FPAIR * (p + 1)])
                    last_u = (p == NP_ - 1)
                    for q in range(NCH // NQG):        # 49 = 7*7 chunks
                        pt = ptp.tile([128, NQG * 128], BF, tag="pt")
                        for jj in range(NQG):
                            m0 = 128 * (NQG * q + jj)
                            nc.tensor.transpose(
                                pt[:, 128 * jj:128 * (jj + 1)],
                                xs[:, m0:m0 + 128], eye[:])
                        xtq = xtqp.tile([128, NQG * 128], BF, tag="xtq")
                        if qeng % 2 == 0:
                            nc.vector.tensor_copy(xtq[:], pt[:])
                        else:
                            nc.scalar.activation(
                                xtq[:], pt[:],
                                mybir.ActivationFunctionType.Copy)
                        qeng += 1
                        for jj in range(NQG):
                            sl = xtq[:, 128 * jj:128 * (jj + 1)]
                            nc.tensor.matmul(
                                cov[h][:], sl, sl,
                                start=not started,
                                stop=(last_u and q == NCH // NQG - 1
                                      and jj == NQG - 1),
                                skip_group_check=True)
                            started = True
                # launch this half's AllReduce as soon as its S2 is done
                nc.vector.tensor_copy(covsb[h][:], cov[h][:])
                nc.gpsimd.dma_start(inb[h][:], covsb[h][:])
                nc.gpsimd.collective_compute(
                    "AllReduce", mybir.AluOpType.add,
                    replica_groups=[list(range(N_CORES))],
                    ins=[inb[h][:].opt()], outs=[outb[h][:].opt()],
                )

            # ------------- Phase 2+3 per half: NS then whiten -------------
            for h in (0, 1):
                # fetch AllReduce result (sync engine has nothing left to
                # issue that could run earlier than this anyway)
                nc.sync.dma_start(s2r[h][:], outb[h][:])
                # sigma = S2 * (mask/M) + (-mu mu^T * mask + eps I)
                sig = nsp.tile([128, 128], FP, name=f"sig{h}")
                nc.vector.tensor_mul(sig[:], s2r[h][:], maskM[:])
                nc.vector.tensor_add(sig[:], sig[:], corr[h][:])
                # Newton-Schulz:  Y0 = sigma, Z0 = I
                # W = 1.5I - 0.5 Z Y;  Y' = Y W;  Z' = W Z   (all symmetric)
                # iter 1 collapses: W1 = 1.5I - 0.5 sigma; Y1 = sig@W1; Z1=W1
                w1 = nsp.tile([128, 128], FP, name=f"w1_{h}")
                nc.vector.tensor_scalar(w1[:], sig[:], -0.5, None,
                                        mybir.AluOpType.mult)
                nc.vector.tensor_add(w1[:], w1[:], i15[:])
                ps = nspsp.tile([128, 128], FP, tag="nsps")
                nc.tensor.matmul(ps[:], sig[:], w1[:])
                ycur = nsp.tile([128, 128], FP, name=f"y1_{h}")
                nc.vector.tensor_copy(ycur[:], ps[:])
                zcur = w1
                for k in range(2, NS_K + 1):
                    pt_ = nspsp.tile([128, 128], FP, tag="nsps")
                    nc.tensor.matmul(pt_[:], zcur[:], ycur[:])
                    wk = nsp.tile([128, 128], FP, name=f"w{k}_{h}")
                    nc.vector.tensor_scalar(wk[:], pt_[:], -0.5, None,
                                            mybir.AluOpType.mult)
                    nc.vector.tensor_add(wk[:], wk[:], i15[:])
                    if k < NS_K:
                        py = nspsp.tile([128, 128], FP, tag="nsps")
                        nc.tensor.matmul(py[:], ycur[:], wk[:])
                        ynew = nsp.tile([128, 128], FP, name=f"y{k}_{h}")
                        nc.vector.tensor_copy(ynew[:], py[:])
                        ycur = ynew
                    pz = nspsp.tile([128, 128], FP, tag="nsps")
                    nc.tensor.matmul(pz[:], wk[:], zcur[:])
                    znew = nsp.tile([128, 128], FP, name=f"z{k}_{h}")
                    nc.vector.tensor_copy(znew[:], pz[:])
                    zcur = znew
                # fold weight in:  wmS[d,c] = wm[d,c] * w[c]  (bf16 for PE)
                psw = nspsp.tile([128, 128], FP, tag="nsps")
                nc.tensor.matmul(psw[:], zcur[:], diagW[h][:])
                wmb = nsp.tile([128, 128], BF, name=f"wmb{h}")
                nc.vector.tensor_copy(wmb[:], psw[:])
                # shift = bias - w * (wm @ mu)  (= bias - wmS^T-applied mu)
                pmv = nspsp.tile([128, 128], FP, tag="nsps")
                nc.tensor.matmul(pmv[:, 0:1], wmb[:], mub[:, h:h + 1])
                shift = nsp.tile([128, 1], FP, name=f"shift{h}")
                nc.vector.tensor_sub(shift[:], bv[:, h:h + 1], pmv[:, 0:1])

                # whiten this half from resident SBUF, stream out in bf16;
                # moves alternate V/S per chunk so both engines drain PSUM
                # concurrently (PE is only wps-bufs ahead of the moves)
                for p in range(NP_):
                    xs = xres[h][:, FPAIR * p:FPAIR * (p + 1)]
                    ot = obufp.tile([128, FPAIR], BF, tag="o")
                    for k in range(NK):
                        ps = wpsp.tile([128, KT], FP, tag="wps")
                        nc.tensor.matmul(ps[:], wmb[:],
                                         xs[:, KT * k:KT * (k + 1)])
                        dst = ot[:, KT * k:KT * (k + 1)]
                        if k % 2 == 0:
                            nc.scalar.activation(
                                dst, ps[:],
                                mybir.ActivationFunctionType.Identity,
                                bias=shift[:, 0:1])
                        else:
                            nc.vector.tensor_scalar(
                                dst, ps[:], shift[:, 0:1], None,
                                mybir.AluOpType.add)
                    nc.sync.dma_start(
                        Xn[128 * h:128 * (h + 1), FPAIR * p:FPAIR * (p + 1)],
                        ot[:])

    nc.compile()
    return nc


_PROGS = {}


def _program():
    if "f" not in _PROGS:
        _PROGS["f"] = _build()
    return _PROGS["f"]


def kernel(X, weight, bias, _return_results=False):
    X = np.asarray(X, dtype=np.float32)
    weight = np.asarray(weight, dtype=np.float32).reshape(C)
    bias = np.asarray(bias, dtype=np.float32).reshape(C)
    nc = _program()

    Xr = X.reshape(N, C, HW)
    mu = Xr.mean(axis=(0, 2), dtype=np.float64)              # exact [256]
    Xc = Xr.transpose(1, 0, 2)                               # [C, N, HW] view
    shards = [np.ascontiguousarray(
        Xc[:, NL * i:NL * (i + 1), :]).reshape(C, MC).astype(ml_dtypes.bfloat16)
        for i in range(N_CORES)]

    mask = np.zeros((128, 128), np.float64)
    for g in range(8):
        mask[CG * g:CG * (g + 1), CG * g:CG * (g + 1)] = 1.0
    maskM = (mask / M_TOT).astype(np.float32)
    corr = np.stack([
        (-np.outer(mu[128 * h:128 * (h + 1)], mu[128 * h:128 * (h + 1)])
         * mask + EPS * np.eye(128)).astype(np.float32)
        for h in (0, 1)])
    i15 = (1.5 * np.eye(128)).astype(np.float32)
    eye = np.eye(128, dtype=ml_dtypes.bfloat16)
    diagW = np.stack([np.diag(weight[128 * h:128 * (h + 1)]).astype(np.float32)
                      for h in (0, 1)])
    mub_in = np.stack([mu[:128], mu[128:]], axis=1).astype(ml_dtypes.bfloat16)
    bv_in = np.stack([bias[:128], bias[128:]], axis=1).astype(np.float32)

    in_maps = [{"X": s, "eye": eye, "maskM": maskM, "corr": corr,
                "i15": i15, "diagW": diagW, "mub": mub_in, "bv": bv_in}
               for s in shards]
    res = run_bass_kernel_spmd(nc, in_maps, list(range(N_CORES)))

    out = np.empty((C, N, HW), np.float32)
    for i, r in enumerate(res.results):
        out[:, NL * i:NL * (i + 1), :] = \
            r["Xn"].reshape(C, NL, HW).astype(np.float32)
    out = np.ascontiguousarray(out.transpose(1, 0, 2)).reshape(N, C, H, W)
    if _return_results:
        return out, res
    return out


# revision 13
# speedup vs baseline: 1.6256x; 1.6256x over previous
"""DBN-Sigma whitening (group-wise decorrelated batch norm), fused
single-pass kernel on 8 trn2 cores.

Strategy (data-parallel over batch N, all-reduce of per-group stats):
  Each core takes 8 of 64 images.  X is host-cast to bf16 in
  channel-major layout ([256, 8*3136] per core, 12.5KB-contiguous DMA
  rows) and streamed into SBUF once, staying RESIDENT (100KB/partition)
  for the whole kernel.  Per 128-channel half: m-chunks are transposed
  ([c,m]->[m,c]) on the PE (bf16 transpose -> PSUM -> V/S copy) and the
  raw second moment S2 = sum_m x x^T accumulates in PSUM via bf16
  matmuls.  S2 of each half is AllReduce'd across the 8 cores (64KB,
  DRAM bounce) as soon as that half's accumulation ends, overlapping
  with the other half's streaming/compute; a tiny warmup AllReduce at
  kernel start absorbs the collective channel's setup latency and
  aligns the cores while the input is still streaming.  sigma is formed
  on device from host-supplied exact-mean correction constants
  (sigma = S2*mask/M - mu mu^T*mask + eps I) and sigma^{-1/2} comes
  from 2 coupled Newton-Schulz iterations in f32 on the PE (sigma's
  eigenvalues are within 2% of 1, so NS matches eigh to ~2e-7 —
  this removes the host eigh of the 2-launch version and with it the
  second read of X).  weight is folded into the whitening matrix on
  device (wm @ diag(w)); whitening runs from resident SBUF data (bf16
  matmuls, bf16 PSUM), the shift (bias - w*wm@mu) is added during the
  PSUM->SBUF move on alternating V/S engines, and the output streams
  out in bf16 channel-major (host upcasts/reorders).

HBM traffic/core: 12.85MB in + 12.85MB out (vs 64MB for the 2-launch
f32 version).  Measured numerics: rel err ~7e-3 (tolerance 2e-2).
"""

import numpy as np
import ml_dtypes
import concourse.bass as bass
import concourse.bacc as bacc
import concourse.mybir as mybir
import concourse.tile as tile
from concourse.bass_utils import run_bass_kernel_spmd

N_CORES = 8
N, C, H, W = 64, 256, 56, 56
HW = H * W                     # 3136
NL = N // N_CORES              # 8 images per core
G, CG = 16, 16
EPS = 1e-3
M_TOT = N * HW
MC = NL * HW                   # 25088 pixels per core
FP = mybir.dt.float32
BF = mybir.dt.bfloat16

NP_ = NL // 2                  # 4 image pairs per core
FPAIR = 2 * HW                 # 6272 free elems per (pair, half) unit
NCH = FPAIR // 128             # 49 m-chunks per unit
NQG = 7                        # transpose chunks per PSUM group (49 = 7*7)
KT = 448                       # whiten matmul free-dim tile (14 * 448 = 6272)
NK = FPAIR // KT
NS_K = 2                       # Newton-Schulz iterations


def _build():
    nc = bacc.Bacc("TRN2", target_bir_lowering=False, debug=False,
                   num_devices=N_CORES)
    X_d = nc.dram_tensor("X", [C, MC], BF, kind="ExternalInput")
    eye_d = nc.dram_tensor("eye", [128, 128], BF, kind="ExternalInput")
    maskM_d = nc.dram_tensor("maskM", [128, 128], FP, kind="ExternalInput")
    corr_d = nc.dram_tensor("corr", [2, 128, 128], FP, kind="ExternalInput")
    i15_d = nc.dram_tensor("i15", [128, 128], FP, kind="ExternalInput")
    diagW_d = nc.dram_tensor("diagW", [2, 128, 128], FP, kind="ExternalInput")
    mub_d = nc.dram_tensor("mub", [128, 2], BF, kind="ExternalInput")
    bv_d = nc.dram_tensor("bv", [128, 2], FP, kind="ExternalInput")
    Xn_d = nc.dram_tensor("Xn", [C, MC], BF, kind="ExternalOutput")
    X = X_d.ap()
    Xn = Xn_d.ap()

    with tile.TileContext(nc) as tc:
        with (
            tc.tile_pool(name="const", bufs=1) as constp,
            tc.tile_pool(name="xres", bufs=1) as xresp,
            tc.tile_pool(name="xtq", bufs=6) as xtqp,
            tc.tile_pool(name="ns", bufs=1) as nsp,
            tc.tile_pool(name="obuf", bufs=3) as obufp,
            tc.tile_pool(name="dram", bufs=1, space="DRAM") as dramp,
        ):
            eye = constp.tile([128, 128], BF)
            nc.sync.dma_start(eye[:], eye_d.ap())
            maskM = constp.tile([128, 128], FP)
            nc.sync.dma_start(maskM[:], maskM_d.ap())
            corr = [constp.tile([128, 128], FP, name=f"corr{h}") for h in (0, 1)]
            for h in (0, 1):
                nc.sync.dma_start(corr[h][:], corr_d.ap()[h])
            i15 = constp.tile([128, 128], FP)
            nc.sync.dma_start(i15[:], i15_d.ap())
            diagW = [constp.tile([128, 128], FP, name=f"diagW{h}")
                     for h in (0, 1)]
            for h in (0, 1):
                nc.sync.dma_start(diagW[h][:], diagW_d.ap()[h])
            mub = constp.tile([128, 2], BF)
            nc.sync.dma_start(mub[:], mub_d.ap())
            bv = constp.tile([128, 2], FP)
            nc.sync.dma_start(bv[:], bv_d.ap())

            xres = [xresp.tile([128, NP_ * FPAIR], BF, name=f"xres{h}")
                    for h in (0, 1)]
            covsb = [nsp.tile([128, 128], FP, name=f"covsb{h}") for h in (0, 1)]
            s2r = [nsp.tile([128, 128], FP, name=f"s2r{h}") for h in (0, 1)]
            inb = [dramp.tile([128, 128], FP, name=f"inb{h}") for h in (0, 1)]
            outb = [dramp.tile([128, 128], FP, name=f"outb{h}") for h in (0, 1)]

            # ---------------- Phase 1: stream in, accumulate S2 ----------
            # PSUM->SBUF copies alternate V/S per chunk-group so both
            # engines drain the PE's transposes concurrently.  The phase-1
            # PSUM pools close before the whiten pools open (8-bank budget).
            p1 = tc.tile_pool(name="ptp", bufs=3, space="PSUM")
            ptp = p1.__enter__()
            p2 = tc.tile_pool(name="cov", bufs=1, space="PSUM")
            covp = p2.__enter__()
            cov = [covp.tile([128, 128], FP, name=f"cov{h}") for h in (0, 1)]
            qeng = 0
            for h in (0, 1):
                started = False
                for p in range(NP_):
                    xs = xres[h][:, FPAIR * p:FPAIR * (p + 1)]
                    nc.sync.dma_start(
                        xs, X[128 * h:128 * (h + 1),
                              FPAIR * p:FPAIR * (p + 1)])
                    last_u = (p == NP_ - 1)
                    for q in range(NCH // NQG):        # 49 = 7*7 chunks
                        pt = ptp.tile([128, NQG * 128], BF, tag="pt")
                        for jj in range(NQG):
                            m0 = 128 * (NQG * q + jj)
                            nc.tensor.transpose(
                                pt[:, 128 * jj:128 * (jj + 1)],
                                xs[:, m0:m0 + 128], eye[:])
                        xtq = xtqp.tile([128, NQG * 128], BF, tag="xtq")
                        if qeng % 2 == 0:
                            nc.vector.tensor_copy(xtq[:], pt[:])
                        else:
                            nc.scalar.activation(
                                xtq[:], pt[:],
                                mybir.ActivationFunctionType.Copy)
                        qeng += 1
                        for jj in range(NQG):
                            sl = xtq[:, 128 * jj:128 * (jj + 1)]
                            nc.tensor.matmul(
                                cov[h][:], sl, sl,
                                start=not started,
                                stop=(last_u and q == NCH // NQG - 1
                                      and jj == NQG - 1),
                                skip_group_check=True)
                            started = True
                # launch this half's AllReduce as soon as its S2 is done
                nc.vector.tensor_copy(covsb[h][:], cov[h][:])
                nc.gpsimd.dma_start(inb[h][:], covsb[h][:])
                nc.gpsimd.collective_compute(
                    "AllReduce", mybir.AluOpType.add,
                    replica_groups=[list(range(N_CORES))],
                    ins=[inb[h][:].opt()], outs=[outb[h][:].opt()],
                )

            p2.__exit__(None, None, None)
            p1.__exit__(None, None, None)
            p3 = tc.tile_pool(name="nsps", bufs=1, space="PSUM")
            nspsp = p3.__enter__()
            p4 = tc.tile_pool(name="wps", bufs=4, space="PSUM")
            wpsp = p4.__enter__()

            # ------------- Phase 2+3 per half: NS then whiten -------------
            for h in (0, 1):
                # fetch AllReduce result (sync engine has nothing left to
                # issue that could run earlier than this anyway)
                nc.sync.dma_start(s2r[h][:], outb[h][:])
                # sigma = S2 * (mask/M) + (-mu mu^T * mask + eps I)
                sig = nsp.tile([128, 128], FP, name=f"sig{h}")
                nc.vector.tensor_mul(sig[:], s2r[h][:], maskM[:])
                nc.vector.tensor_add(sig[:], sig[:], corr[h][:])
                # Newton-Schulz:  Y0 = sigma, Z0 = I
                # W = 1.5I - 0.5 Z Y;  Y' = Y W;  Z' = W Z   (all symmetric)
                # iter 1 collapses: W1 = 1.5I - 0.5 sigma; Y1 = sig@W1; Z1=W1
                w1 = nsp.tile([128, 128], FP, name=f"w1_{h}")
                nc.vector.tensor_scalar(w1[:], sig[:], -0.5, None,
                                        mybir.AluOpType.mult)
                nc.vector.tensor_add(w1[:], w1[:], i15[:])
                ps = nspsp.tile([128, 128], FP, tag="nsps")
                nc.tensor.matmul(ps[:], sig[:], w1[:])
                ycur = nsp.tile([128, 128], FP, name=f"y1_{h}")
                nc.vector.tensor_copy(ycur[:], ps[:])
                zcur = w1
                for k in range(2, NS_K + 1):
                    pt_ = nspsp.tile([128, 128], FP, tag="nsps")
                    nc.tensor.matmul(pt_[:], zcur[:], ycur[:])
                    wk = nsp.tile([128, 128], FP, name=f"w{k}_{h}")
                    nc.vector.tensor_scalar(wk[:], pt_[:], -0.5, None,
                                            mybir.AluOpType.mult)
                    nc.vector.tensor_add(wk[:], wk[:], i15[:])
                    if k < NS_K:
                        py = nspsp.tile([128, 128], FP, tag="nsps")
                        nc.tensor.matmul(py[:], ycur[:], wk[:])
                        ynew = nsp.tile([128, 128], FP, name=f"y{k}_{h}")
                        nc.vector.tensor_copy(ynew[:], py[:])
                        ycur = ynew
                    pz = nspsp.tile([128, 128], FP, tag="nsps")
                    nc.tensor.matmul(pz[:], wk[:], zcur[:])
                    znew = nsp.tile([128, 128], FP, name=f"z{k}_{h}")
                    nc.vector.tensor_copy(znew[:], pz[:])
                    zcur = znew
                # fold weight in:  wmS[d,c] = wm[d,c] * w[c]  (bf16 for PE)
                psw = nspsp.tile([128, 128], FP, tag="nsps")
                nc.tensor.matmul(psw[:], zcur[:], diagW[h][:])
                wmb = nsp.tile([128, 128], BF, name=f"wmb{h}")
                nc.vector.tensor_copy(wmb[:], psw[:])
                # shift = bias - w * (wm @ mu)  (= bias - wmS^T-applied mu)
                pmv = nspsp.tile([128, 128], FP, tag="nsps")
                nc.tensor.matmul(pmv[:, 0:1], wmb[:], mub[:, h:h + 1])
                shift = nsp.tile([128, 1], FP, name=f"shift{h}")
                nc.vector.tensor_sub(shift[:], bv[:, h:h + 1], pmv[:, 0:1])

                # whiten this half from resident SBUF, stream out in bf16;
                # moves alternate V/S per chunk so both engines drain PSUM
                # concurrently (PE is only wps-bufs ahead of the moves)
                for p in range(NP_):
                    xs = xres[h][:, FPAIR * p:FPAIR * (p + 1)]
                    ot = obufp.tile([128, FPAIR], BF, tag="o")
                    for k in range(NK):
                        ps = wpsp.tile([128, KT], FP, tag="wps")
                        nc.tensor.matmul(ps[:], wmb[:],
                                         xs[:, KT * k:KT * (k + 1)])
                        dst = ot[:, KT * k:KT * (k + 1)]
                        if k % 2 == 0:
                            nc.scalar.activation(
                                dst, ps[:],
                                mybir.ActivationFunctionType.Identity,
                                bias=shift[:, 0:1])
                        else:
                            nc.vector.tensor_scalar(
                                dst, ps[:], shift[:, 0:1], None,
                                mybir.AluOpType.add)
                    nc.sync.dma_start(
                        Xn[128 * h:128 * (h + 1), FPAIR * p:FPAIR * (p + 1)],
                        ot[:])

            p4.__exit__(None, None, None)
            p3.__exit__(None, None, None)

    nc.compile()
    return nc


_PROGS = {}


def _program():
    if "f" not in _PROGS:
        _PROGS["f"] = _build()
    return _PROGS["f"]


def kernel(X, weight, bias, _return_results=False):
    X = np.asarray(X, dtype=np.float32)
    weight = np.asarray(weight, dtype=np.float32).reshape(C)
    bias = np.asarray(bias, dtype=np.float32).reshape(C)
    nc = _program()

    Xr = X.reshape(N, C, HW)
    mu = Xr.mean(axis=(0, 2), dtype=np.float64)              # exact [256]
    Xc = Xr.transpose(1, 0, 2)                               # [C, N, HW] view
    shards = [np.ascontiguousarray(
        Xc[:, NL * i:NL * (i + 1), :]).reshape(C, MC).astype(ml_dtypes.bfloat16)
        for i in range(N_CORES)]

    mask = np.zeros((128, 128), np.float64)
    for g in range(8):
        mask[CG * g:CG * (g + 1), CG * g:CG * (g + 1)] = 1.0
    maskM = (mask / M_TOT).astype(np.float32)
    corr = np.stack([
        (-np.outer(mu[128 * h:128 * (h + 1)], mu[128 * h:128 * (h + 1)])
         * mask + EPS * np.eye(128)).astype(np.float32)
        for h in (0, 1)])
    i15 = (1.5 * np.eye(128)).astype(np.float32)
    eye = np.eye(128, dtype=ml_dtypes.bfloat16)
    diagW = np.stack([np.diag(weight[128 * h:128 * (h + 1)]).astype(np.float32)
                      for h in (0, 1)])
    mub_in = np.stack([mu[:128], mu[128:]], axis=1).astype(ml_dtypes.bfloat16)
    bv_in = np.stack([bias[:128], bias[128:]], axis=1).astype(np.float32)

    in_maps = [{"X": s, "eye": eye, "maskM": maskM, "corr": corr,
                "i15": i15, "diagW": diagW, "mub": mub_in, "bv": bv_in}
               for s in shards]
    res = run_bass_kernel_spmd(nc, in_maps, list(range(N_CORES)))

    out = np.empty((C, N, HW), np.float32)
    for i, r in enumerate(res.results):
        out[:, NL * i:NL * (i + 1), :] = \
            r["Xn"].reshape(C, NL, HW).astype(np.float32)
    out = np.ascontiguousarray(out.transpose(1, 0, 2)).reshape(N, C, H, W)
    if _return_results:
        return out, res
    return out
